# revision 17
# baseline (speedup 1.0000x reference)
"""Trainium2 Bass kernel for nn_AttentionBlock (B=4, H=W=64, C=256, D=32).

Sharding: 8 shards = 4 samples x 2 query-halves. Host pre-transposes x to
channel-major (xT) per core, so the kernel starts straight into the
projections. Each core computes K/V for all 4096 keys and attention +
output projection for its 2048 queries. Residual and bias folds are done
on host (exact, f32). No collectives.

Device structure per core:
  - q/k/v projections from xT (kT packed into 2 row-strips for 2x
    row-tiled S matmuls; qT replicated on partitions 0-63).
  - 4 supergroups of 512 queries; per supergroup 16 "sets" of 2 key
    chunks: S = kT^T @ qT via two concurrent 32x128 row-tiled matmuls,
    exp on ACT (scaled by 2^-4, cancels in normalization), attend
    accumulated into per-128-query psum tiles with a ones-column for the
    softmax denominator, software-pipelined one set behind S/exp.
  - epilogue: normalize, transpose via PE, output projection, staged
    store (one DMA per supergroup).

PSUM: pool "ps" tag ps [128,1024] f32 x2 bufs = 4 banks (S sets + phase B),
pool "ps_att" tag a [128,512] f32 x4 bufs = 4 banks (pa accumulators +
epilogue transpose/out-proj scratch). Total 8 banks.

Self-contained: hardcodes shapes, imports only /opt/trn_rl_repo concourse.
"""

import sys

if "/opt/trn_rl_repo" not in sys.path:
    sys.path.insert(0, "/opt/trn_rl_repo")

import numpy as np
import ml_dtypes

BF16 = ml_dtypes.bfloat16

# Problem constants
B, HH, WW, C = 4, 64, 64, 256
D = 32
N = HH * WW           # 4096 keys per sample
NQ = N // 2           # 2048 queries per core
NCORES = 8
KC = N // 128         # 32 key chunks
NSETS = KC // 2       # 16 sets of 2 chunks
NSG = NQ // 512       # 4 supergroups of 512 queries
EXP_BIAS = float(-4.0 * np.log(2.0))  # et = exp(s) * 2^-4 (cancels in softmax)

_compiled_cache = {}


def _build(use_bias: bool):
    from contextlib import ExitStack
    from concourse import bacc, tile, mybir, masks

    f32 = mybir.dt.float32
    bf = mybir.dt.bfloat16

    nc = bacc.Bacc("TRN2", target_bir_lowering=False, debug=False, num_devices=NCORES)

    VW = 257 if use_bias else 256  # v-proj output cols (col 256 = r for bias fold)

    xT_d = nc.dram_tensor("xT16", [2, 128, N], bf, kind="ExternalInput")
    wqp_d = nc.dram_tensor("wqp", [2, 128, 64], bf, kind="ExternalInput")
    wkp_d = nc.dram_tensor("wkp", [2, 128, 64], bf, kind="ExternalInput")
    wvp_d = nc.dram_tensor("wvp", [2, 128, VW], bf, kind="ExternalInput")
    wop_d = nc.dram_tensor("wop", [2, 128, 256], bf, kind="ExternalInput")
    out_d = nc.dram_tensor("out", [NQ, C], f32, kind="ExternalOutput")

    Exp = mybir.ActivationFunctionType.Exp
    Mult = mybir.AluOpType.mult

    with tile.TileContext(nc) as tc:
        with ExitStack() as ctx:
            const = ctx.enter_context(tc.tile_pool(name="const", bufs=1))
            expp = ctx.enter_context(tc.tile_pool(name="expp", bufs=4))
            small = ctx.enter_context(tc.tile_pool(name="small", bufs=3))
            ps = ctx.enter_context(tc.tile_pool(name="ps", bufs=2, space="PSUM"))
            ps_att = ctx.enter_context(tc.tile_pool(name="ps_att", bufs=4, space="PSUM"))

            # ---- constants & weights ----
            ident = const.tile([128, 128], bf, tag="ident")
            masks.make_identity(nc, ident[:])
            ebias = const.tile([128, 1], f32, tag="ebias")
            nc.gpsimd.memset(ebias[:], EXP_BIAS)

            wq0 = const.tile([128, 64], bf, tag="wq0")
            wq1 = const.tile([128, 64], bf, tag="wq1")
            wk0 = const.tile([128, 64], bf, tag="wk0")
            wk1 = const.tile([128, 64], bf, tag="wk1")
            wv0 = const.tile([128, VW], bf, tag="wv0")
            wv1 = const.tile([128, VW], bf, tag="wv1")
            wo0 = const.tile([128, 256], bf, tag="wo0")
            wo1 = const.tile([128, 256], bf, tag="wo1")
            nc.gpsimd.dma_start(out=wq0[:], in_=wqp_d[0, :, :])
            nc.gpsimd.dma_start(out=wq1[:], in_=wqp_d[1, :, :])
            nc.gpsimd.dma_start(out=wk0[:], in_=wkp_d[0, :, :])
            nc.gpsimd.dma_start(out=wk1[:], in_=wkp_d[1, :, :])
            nc.gpsimd.dma_start(out=wv0[:], in_=wvp_d[0, :, :])
            nc.gpsimd.dma_start(out=wv1[:], in_=wvp_d[1, :, :])
            nc.gpsimd.dma_start(out=wo0[:], in_=wop_d[0, :, :])
            nc.gpsimd.dma_start(out=wo1[:], in_=wop_d[1, :, :])

            # ---- xT load: [128, 2, 4096], first-half tokens first ----
            xT = const.tile([128, 2, N], bf, tag="xT")
            for j in range(4):
                sl = slice(1024 * j, 1024 * j + 1024)
                nc.sync.dma_start(out=xT[:, 0, sl], in_=xT_d[0, :, sl])
                nc.scalar.dma_start(out=xT[:, 1, sl], in_=xT_d[1, :, sl])

            # SBUF destinations
            qT = const.tile([64, NQ], bf, tag="qT")           # 2 replicas of q^T
            kT = const.tile([64, NSETS * 128], bf, tag="kT")  # strip i = chunks 2t+i
            vsb = const.tile([128, KC, 260], bf, tag="vsb")
            stage = const.tile([128, 16, 256], f32, tag="stage")
            nc.gpsimd.memset(vsb[:, :, 256:257], 1.0)

            def q_proj(j):  # 512-token chunk j of the 2048 queries
                pq = ps.tile([128, 1024], f32, tag="ps", name=f"pq{j}")
                sl = slice(512 * j, 512 * j + 512)
                nc.tensor.matmul(pq[0:64, 0:512], wq0[:], xT[:, 0, sl], start=True, stop=False)
                nc.tensor.matmul(pq[0:64, 0:512], wq1[:], xT[:, 1, sl], start=False, stop=True)
                nc.vector.tensor_copy(qT[:, sl], pq[0:64, 0:512])

            def k_proj(s):  # 512-token chunk s of all 4096 keys
                pk = ps.tile([128, 1024], f32, tag="ps", name=f"pk{s}")
                sl = slice(512 * s, 512 * s + 512)
                nc.tensor.matmul(pk[0:64, 0:512], wk0[:], xT[:, 0, sl], start=True, stop=False)
                nc.tensor.matmul(pk[0:64, 0:512], wk1[:], xT[:, 1, sl], start=False, stop=True)
                # strip-pack: even global chunks -> partitions 0:32, odd -> 32:64
                # within-tile chunks j=0..3 (tokens 128j): j=2a+b, b=0 even.
                dst = slice(256 * s, 256 * s + 256)
                src0 = pk[0:32, 0:512].rearrange("p (a b t) -> p a b t", a=2, b=2)
                src1 = pk[32:64, 0:512].rearrange("p (a b t) -> p a b t", a=2, b=2)
                nc.vector.tensor_copy(
                    kT[0:32, dst].rearrange("p (a t) -> p a t", a=2), src0[:, :, 0, :]
                )
                nc.vector.tensor_copy(
                    kT[32:64, dst].rearrange("p (a t) -> p a t", a=2), src1[:, :, 1, :]
                )

            def v_proj(m2):  # pair of 128-token chunks (2*m2, 2*m2+1)
                pv = ps.tile([128, 1024], f32, tag="ps", name=f"pv{m2}")
                for r in range(2):
                    m = 2 * m2 + r
                    osl = pv[:, 512 * r : 512 * r + VW]
                    tsl = slice(128 * m, 128 * m + 128)
                    nc.tensor.matmul(osl, xT[:, 0, tsl], wv0[:], start=True, stop=False)
                    nc.tensor.matmul(osl, xT[:, 1, tsl], wv1[:], start=False, stop=True)
                for r in range(2):
                    m = 2 * m2 + r
                    osl = pv[:, 512 * r : 512 * r + VW]
                    if use_bias:
                        rv = small.tile([128, 1], f32, tag="rv")
                        nc.scalar.activation(rv[:], osl[:, 256:257], Exp)
                        nc.vector.tensor_scalar(vsb[:, m, 0:256], osl[:, 0:256], rv[:], None, Mult)
                        nc.vector.tensor_copy(vsb[:, m, 256:257], rv[:])
                    else:
                        nc.vector.tensor_copy(vsb[:, m, 0:256], osl[:, 0:256])

            def epilogue_batch(g, pas):
                # all-DVE normalize first, then stream PE work; the freed pa
                # tiles double as transpose/out-proj psum scratch.
                ats = []
                for qh in range(4):
                    rec = small.tile([128, 1], f32, tag="rec", bufs=4)
                    nc.vector.reciprocal(rec[:], pas[qh][:, 256:257])
                    at = small.tile([128, 256], bf, tag="at", bufs=4)
                    nc.vector.tensor_scalar(at[:], pas[qh][:, 0:256], rec[:], None, Mult)
                    ats.append(at)
                for qh in range(4):
                    pa, at = pas[qh], ats[qh]
                    nc.tensor.matmul(pa[:, 256:384], at[:, 0:128], ident[:], start=True, stop=True)
                    nc.tensor.matmul(pa[:, 384:512], at[:, 128:256], ident[:], start=True, stop=True)
                aTs = []
                for qh in range(4):
                    aT = small.tile([128, 256], bf, tag="aT", bufs=4)
                    nc.vector.tensor_copy(aT[:], pas[qh][:, 256:512])
                    aTs.append(aT)
                for qh in range(4):
                    pa, aT = pas[qh], aTs[qh]
                    nc.tensor.matmul(pa[:, 0:256], aT[:, 0:128], wo0[:, 0:256], start=True, stop=False)
                    nc.tensor.matmul(pa[:, 0:256], aT[:, 128:256], wo1[:, 0:256], start=False, stop=True)
                for qh in range(4):
                    nc.vector.tensor_copy(stage[:, 4 * g + qh, :], pas[qh][:, 0:256])
                nc.sync.dma_start(
                    out=out_d[:].rearrange("(t p) c -> p t c", p=128)[:, 4 * g : 4 * g + 4, :],
                    in_=stage[:, 4 * g : 4 * g + 4, :],
                )

            # ---- head: k first (gates S), q0, first two v chunks ----
            k_proj(0)
            k_proj(1)
            q_proj(0)
            for s in range(2, 8):
                k_proj(s)
            v_proj(0)

            # ---- phase C: S -> exp -> attend pipelined 2 sets deep ----
            pa_tiles = {}
            ets = {}  # si -> (et, g, t)
            epi_pending = None

            total = NSG * NSETS
            for si in range(total + 2):
                pst = None
                if si < total:
                    g, t = divmod(si, NSETS)
                    qsl = slice(512 * g, 512 * g + 512)
                    pst = ps.tile([128, 1024], f32, tag="ps", name=f"pst{si}")
                    nc.tensor.matmul(
                        pst[:, 0:512], kT[0:32, 128 * t : 128 * t + 128],
                        qT[0:32, qsl], start=True, stop=True, tile_position=(0, 0),
                    )
                    nc.tensor.matmul(
                        pst[:, 512:1024], kT[32:64, 128 * t : 128 * t + 128],
                        qT[32:64, qsl], start=True, stop=True, tile_position=(32, 0),
                    )
                    # interleaved remaining projections during supergroup 0
                    if g == 0 and t < 15:
                        v_proj(t + 1)
                    if t == 12 and g < NSG - 1:
                        q_proj(g + 1)
                # attend two sets behind S/exp
                if si - 2 in ets:
                    et_p, g_p, t_p = ets.pop(si - 2)
                    if t_p == 0:
                        for qh in range(4):
                            pa_tiles[(g_p, qh)] = ps_att.tile(
                                [128, 512], f32, tag="a", name=f"pa{g_p}_{qh}"
                            )
                    for qh in range(4):
                        pa = pa_tiles[(g_p, qh)]
                        for ci in range(2):
                            nc.tensor.matmul(
                                pa[:, 0:257],
                                et_p[:, 512 * ci + 128 * qh : 512 * ci + 128 * qh + 128],
                                vsb[:, 2 * t_p + ci, 0:257],
                                start=(t_p == 0 and ci == 0),
                                stop=(t_p == NSETS - 1 and ci == 1),
                            )
                    if t_p == NSETS - 1:
                        epi_pending = g_p
                if si < total:
                    et = expp.tile([128, 1024], bf, tag="e")
                    nc.scalar.activation(et[:], pst[:], Exp, bias=ebias[:])
                    ets[si] = (et, g, t)
                # epilogue after this slot's exp
                if epi_pending is not None:
                    epilogue_batch(
                        epi_pending,
                        [pa_tiles.pop((epi_pending, qh)) for qh in range(4)],
                    )
                    epi_pending = None

    nc.compile()
    return nc


def _get_compiled(use_bias: bool):
    key = bool(use_bias)
    if key not in _compiled_cache:
        _compiled_cache[key] = _build(use_bias)
    return _compiled_cache[key]


def _prep(x, wq, bq, wk, bk, wv, bv, wo, bo):
    xf = np.ascontiguousarray(np.asarray(x, dtype=np.float32)).reshape(B, N, C)
    wq = np.asarray(wq, np.float32)
    bq = np.asarray(bq, np.float32)
    wk = np.asarray(wk, np.float32)
    bk = np.asarray(bk, np.float32)
    wv = np.asarray(wv, np.float32)
    bv = np.asarray(bv, np.float32)
    wo = np.asarray(wo, np.float32)
    bo = np.asarray(bo, np.float32)

    use_bias = not (np.all(bq == 0) and np.all(bk == 0) and np.all(bv == 0))

    scale = np.float32(1.0 / np.sqrt(np.float32(D)))
    wqs = wq * scale
    # lhsT tiles: [2 c-halves, 128, 64] with d replicated 2x along columns
    wqp = np.ascontiguousarray(np.tile(wqs.reshape(2, 128, D), (1, 1, 2))).astype(BF16)
    wkp = np.ascontiguousarray(np.tile(wk.reshape(2, 128, D), (1, 1, 2))).astype(BF16)
    if use_bias:
        # scores row-fold: r_k = x_k @ u, u = scale * (wk @ bq); exp(r) scales
        # key k's et column (bk and bq*bk terms drop out of softmax).
        u = (wk @ (bq * scale)).astype(np.float32)  # [C]
        wvx = np.concatenate([wv, u[:, None]], axis=1)  # [C, 257]
        wvp = np.ascontiguousarray(wvx.reshape(2, 128, 257)).astype(BF16)
    else:
        wvp = np.ascontiguousarray(wv.reshape(2, 128, 256)).astype(BF16)
    wop = np.ascontiguousarray(wo.reshape(2, 128, 256)).astype(BF16)

    in_maps = []
    for core in range(NCORES):
        b, h = divmod(core, 2)
        if h == 0:
            xo = xf[b]
        else:
            xo = np.concatenate([xf[b, NQ:], xf[b, :NQ]], 0)
        xT = np.ascontiguousarray(xo.T.reshape(2, 128, N)).astype(BF16)
        in_maps.append(
            {"xT16": xT, "wqp": wqp, "wkp": wkp, "wvp": wvp, "wop": wop}
        )
    # host residual fold: out += x + (bv @ wo + bo)
    resid_const = (bv.astype(np.float64) @ wo.astype(np.float64)).astype(np.float32) + bo
    return in_maps, use_bias, xf, resid_const


def _gather(results, xf, resid_const):
    out = np.empty((B, N, C), np.float32)
    for core in range(NCORES):
        b, h = divmod(core, 2)
        out[b, NQ * h : NQ * (h + 1)] = results[core]["out"]
    out += xf
    out += resid_const[None, None, :]
    return out.reshape(B, HH, WW, C)


def kernel(x, wq, bq, wk, bk, wv, bv, wo, bo):
    from concourse.bass_utils import run_bass_kernel_spmd

    in_maps, use_bias, xf, resid_const = _prep(x, wq, bq, wk, bk, wv, bv, wo, bo)
    nc = _get_compiled(use_bias)
    res = run_bass_kernel_spmd(nc, in_maps, core_ids=list(range(NCORES)))
    return _gather(res.results, xf, resid_const)


def _ensure_ntff_hook():
    """The agent image's antenv stub lacks axon_hooks; synthesize it so
    run_bass_kernel_spmd(trace=True) can NTFF-profile via libaxon_pjrt."""
    import types

    try:
        from antenv.axon_hooks import get_axon_ntff_profile_hook  # noqa: F401
        return
    except ImportError:
        pass
    import antenv
    from trn_agent_boot.trn_boot import _ntff_profile_via_ctypes

    mod = types.ModuleType("antenv.axon_hooks")
    state = {"h": _ntff_profile_via_ctypes("/opt/axon/libaxon_pjrt.so")}
    mod.get_axon_ntff_profile_hook = lambda: state["h"]
    mod.set_axon_ntff_profile_hook = lambda h: state.__setitem__("h", h)
    sys.modules["antenv.axon_hooks"] = mod
    antenv.axon_hooks = mod


def run_traced(inputs, **kw):
    """For test.py: run with NTFF profiling; returns (output, BassKernelResults)."""
    from concourse.bass_utils import run_bass_kernel_spmd

    _ensure_ntff_hook()

    in_maps, use_bias, xf, resid_const = _prep(**inputs)
    nc = _get_compiled(use_bias)
    res = run_bass_kernel_spmd(nc, in_maps, core_ids=list(range(NCORES)), trace=True, **kw)
    return _gather(res.results, xf, resid_const), res


# revision 20
# speedup vs baseline: 1.1717x; 1.1717x over previous
"""Trainium2 Bass kernel for nn_AttentionBlock (B=4, H=W=64, C=256, D=32).

Sharding: 8 shards = 4 samples x 2 query-halves. Host pre-transposes x to
channel-major (xT) per core, so the kernel starts straight into the
projections. Each core computes K/V for all 4096 keys and attention +
output projection for its 2048 queries. Residual and bias folds are done
on host (exact, f32). No collectives.

Device structure per core:
  - q/k/v projections from xT (kT packed into 2 row-strips for 2x
    row-tiled S matmuls; qT replicated on partitions 0-63).
  - 4 supergroups of 512 queries; per supergroup 16 "sets" of 2 key
    chunks: S = kT^T @ qT via two concurrent 32x128 row-tiled matmuls,
    exp on ACT (scaled by 2^-4, cancels in normalization), attend
    accumulated into per-128-query psum tiles with a ones-column for the
    softmax denominator, software-pipelined one set behind S/exp.
  - epilogue: normalize, transpose via PE, output projection, staged
    store (one DMA per supergroup).

PSUM: pool "ps" tag ps [128,1024] f32 x2 bufs = 4 banks (S sets + phase B),
pool "ps_att" tag a [128,512] f32 x4 bufs = 4 banks (pa accumulators +
epilogue transpose/out-proj scratch). Total 8 banks.

Self-contained: hardcodes shapes, imports only /opt/trn_rl_repo concourse.
"""

import sys

if "/opt/trn_rl_repo" not in sys.path:
    sys.path.insert(0, "/opt/trn_rl_repo")

import numpy as np
import ml_dtypes

BF16 = ml_dtypes.bfloat16

# Problem constants
B, HH, WW, C = 4, 64, 64, 256
D = 32
N = HH * WW           # 4096 keys per sample
NQ = N // 2           # 2048 queries per core
NCORES = 8
KC = N // 128         # 32 key chunks
NSETS = KC // 2       # 16 sets of 2 chunks
NSG = NQ // 512       # 4 supergroups of 512 queries
EXP_BIAS = float(-4.0 * np.log(2.0))  # et = exp(s) * 2^-4 (cancels in softmax)
USE_FP8 = True  # fp8e4 et/vsb + DoubleRow attend (2 key chunks per matmul)

_compiled_cache = {}


def _build(use_bias: bool):
    from contextlib import ExitStack
    from concourse import bacc, tile, mybir, masks

    f32 = mybir.dt.float32
    bf = mybir.dt.bfloat16
    fp8 = mybir.dt.float8e4
    edt = fp8 if USE_FP8 else bf
    VSW = 272 if USE_FP8 else 260  # vsb row stride (DR rhs needs %16 bytes)

    nc = bacc.Bacc("TRN2", target_bir_lowering=False, debug=False, num_devices=NCORES)

    VW = 257 if use_bias else 256  # v-proj output cols (col 256 = r for bias fold)

    xT_d = nc.dram_tensor("xT16", [2, 128, N], bf, kind="ExternalInput")
    wqp_d = nc.dram_tensor("wqp", [2, 128, 64], bf, kind="ExternalInput")
    wkp_d = nc.dram_tensor("wkp", [2, 128, 64], bf, kind="ExternalInput")
    wvp_d = nc.dram_tensor("wvp", [2, 128, VW], bf, kind="ExternalInput")
    wop_d = nc.dram_tensor("wop", [2, 128, 256], bf, kind="ExternalInput")
    out_d = nc.dram_tensor("out", [NQ, C], f32, kind="ExternalOutput")

    Exp = mybir.ActivationFunctionType.Exp
    Mult = mybir.AluOpType.mult

    with tile.TileContext(nc) as tc:
        with ExitStack() as ctx:
            const = ctx.enter_context(tc.tile_pool(name="const", bufs=1))
            expp = ctx.enter_context(tc.tile_pool(name="expp", bufs=4))
            small = ctx.enter_context(tc.tile_pool(name="small", bufs=3))
            ps = ctx.enter_context(tc.tile_pool(name="ps", bufs=2, space="PSUM"))
            ps_att = ctx.enter_context(tc.tile_pool(name="ps_att", bufs=4, space="PSUM"))

            # ---- constants & weights ----
            ident = const.tile([128, 128], bf, tag="ident")
            masks.make_identity(nc, ident[:])
            ebias = const.tile([128, 1], f32, tag="ebias")
            nc.gpsimd.memset(ebias[:], EXP_BIAS)

            wq0 = const.tile([128, 64], bf, tag="wq0")
            wq1 = const.tile([128, 64], bf, tag="wq1")
            wk0 = const.tile([128, 64], bf, tag="wk0")
            wk1 = const.tile([128, 64], bf, tag="wk1")
            wv0 = const.tile([128, VW], bf, tag="wv0")
            wv1 = const.tile([128, VW], bf, tag="wv1")
            wo0 = const.tile([128, 256], bf, tag="wo0")
            wo1 = const.tile([128, 256], bf, tag="wo1")
            nc.gpsimd.dma_start(out=wq0[:], in_=wqp_d[0, :, :])
            nc.gpsimd.dma_start(out=wq1[:], in_=wqp_d[1, :, :])
            nc.gpsimd.dma_start(out=wk0[:], in_=wkp_d[0, :, :])
            nc.gpsimd.dma_start(out=wk1[:], in_=wkp_d[1, :, :])
            nc.gpsimd.dma_start(out=wv0[:], in_=wvp_d[0, :, :])
            nc.gpsimd.dma_start(out=wv1[:], in_=wvp_d[1, :, :])
            nc.gpsimd.dma_start(out=wo0[:], in_=wop_d[0, :, :])
            nc.gpsimd.dma_start(out=wo1[:], in_=wop_d[1, :, :])

            # ---- xT load: [128, 2, 4096], first-half tokens first ----
            xT = const.tile([128, 2, N], bf, tag="xT")
            for j in range(4):
                sl = slice(1024 * j, 1024 * j + 1024)
                nc.sync.dma_start(out=xT[:, 0, sl], in_=xT_d[0, :, sl])
                nc.scalar.dma_start(out=xT[:, 1, sl], in_=xT_d[1, :, sl])

            # SBUF destinations
            qT = const.tile([64, NQ], bf, tag="qT")           # 2 replicas of q^T
            kT = const.tile([64, NSETS * 128], bf, tag="kT")  # strip i = chunks 2t+i
            vsb = const.tile([128, KC, VSW], edt, tag="vsb")
            stage = const.tile([128, 16, 256], f32, tag="stage")
            nc.gpsimd.memset(vsb[:, :, 256:257], 1.0)

            def q_proj(j):  # 512-token chunk j of the 2048 queries
                pq = ps.tile([128, 1024], f32, tag="ps", name=f"pq{j}")
                sl = slice(512 * j, 512 * j + 512)
                nc.tensor.matmul(pq[0:64, 0:512], wq0[:], xT[:, 0, sl], start=True, stop=False)
                nc.tensor.matmul(pq[0:64, 0:512], wq1[:], xT[:, 1, sl], start=False, stop=True)
                nc.vector.tensor_copy(qT[:, sl], pq[0:64, 0:512])

            def k_proj(s):  # 512-token chunk s of all 4096 keys
                pk = ps.tile([128, 1024], f32, tag="ps", name=f"pk{s}")
                sl = slice(512 * s, 512 * s + 512)
                nc.tensor.matmul(pk[0:64, 0:512], wk0[:], xT[:, 0, sl], start=True, stop=False)
                nc.tensor.matmul(pk[0:64, 0:512], wk1[:], xT[:, 1, sl], start=False, stop=True)
                # strip-pack: even global chunks -> partitions 0:32, odd -> 32:64
                # within-tile chunks j=0..3 (tokens 128j): j=2a+b, b=0 even.
                dst = slice(256 * s, 256 * s + 256)
                src0 = pk[0:32, 0:512].rearrange("p (a b t) -> p a b t", a=2, b=2)
                src1 = pk[32:64, 0:512].rearrange("p (a b t) -> p a b t", a=2, b=2)
                nc.vector.tensor_copy(
                    kT[0:32, dst].rearrange("p (a t) -> p a t", a=2), src0[:, :, 0, :]
                )
                nc.vector.tensor_copy(
                    kT[32:64, dst].rearrange("p (a t) -> p a t", a=2), src1[:, :, 1, :]
                )

            def v_proj(m2):  # pair of 128-token chunks (2*m2, 2*m2+1)
                pv = ps.tile([128, 1024], f32, tag="ps", name=f"pv{m2}")
                for r in range(2):
                    m = 2 * m2 + r
                    osl = pv[:, 512 * r : 512 * r + VW]
                    tsl = slice(128 * m, 128 * m + 128)
                    nc.tensor.matmul(osl, xT[:, 0, tsl], wv0[:], start=True, stop=False)
                    nc.tensor.matmul(osl, xT[:, 1, tsl], wv1[:], start=False, stop=True)
                for r in range(2):
                    m = 2 * m2 + r
                    osl = pv[:, 512 * r : 512 * r + VW]
                    if use_bias:
                        rv = small.tile([128, 1], f32, tag="rv")
                        nc.scalar.activation(rv[:], osl[:, 256:257], Exp)
                        nc.vector.tensor_scalar(vsb[:, m, 0:256], osl[:, 0:256], rv[:], None, Mult)
                        nc.vector.tensor_copy(vsb[:, m, 256:257], rv[:])
                    else:
                        nc.vector.tensor_copy(vsb[:, m, 0:256], osl[:, 0:256])

            def epilogue_batch(g, pas):
                # all-DVE normalize first, then stream PE work; the freed pa
                # tiles double as transpose/out-proj psum scratch.
                ats = []
                for qh in range(4):
                    rec = small.tile([128, 1], f32, tag="rec", bufs=4)
                    nc.vector.reciprocal(rec[:], pas[qh][:, 256:257])
                    at = small.tile([128, 256], bf, tag="at", bufs=4)
                    nc.vector.tensor_scalar(at[:], pas[qh][:, 0:256], rec[:], None, Mult)
                    ats.append(at)
                for qh in range(4):
                    pa, at = pas[qh], ats[qh]
                    nc.tensor.matmul(pa[:, 256:384], at[:, 0:128], ident[:], start=True, stop=True)
                    nc.tensor.matmul(pa[:, 384:512], at[:, 128:256], ident[:], start=True, stop=True)
                aTs = []
                for qh in range(4):
                    aT = small.tile([128, 256], bf, tag="aT", bufs=4)
                    nc.vector.tensor_copy(aT[:], pas[qh][:, 256:512])
                    aTs.append(aT)
                for qh in range(4):
                    pa, aT = pas[qh], aTs[qh]
                    nc.tensor.matmul(pa[:, 0:256], aT[:, 0:128], wo0[:, 0:256], start=True, stop=False)
                    nc.tensor.matmul(pa[:, 0:256], aT[:, 128:256], wo1[:, 0:256], start=False, stop=True)
                for qh in range(4):
                    nc.vector.tensor_copy(stage[:, 4 * g + qh, :], pas[qh][:, 0:256])
                nc.sync.dma_start(
                    out=out_d[:].rearrange("(t p) c -> p t c", p=128)[:, 4 * g : 4 * g + 4, :],
                    in_=stage[:, 4 * g : 4 * g + 4, :],
                )

            # ---- head: k first (gates S), q0, first two v chunks ----
            k_proj(0)
            k_proj(1)
            q_proj(0)
            for s in range(2, 8):
                k_proj(s)
            v_proj(0)

            # ---- phase C: S -> exp -> attend pipelined 2 sets deep ----
            pa_tiles = {}
            ets = {}  # si -> (et, g, t)
            epi_pending = None

            total = NSG * NSETS
            for si in range(total + 2):
                pst = None
                if si < total:
                    g, t = divmod(si, NSETS)
                    qsl = slice(512 * g, 512 * g + 512)
                    pst = ps.tile([128, 1024], f32, tag="ps", name=f"pst{si}")
                    nc.tensor.matmul(
                        pst[:, 0:512], kT[0:32, 128 * t : 128 * t + 128],
                        qT[0:32, qsl], start=True, stop=True, tile_position=(0, 0),
                    )
                    nc.tensor.matmul(
                        pst[:, 512:1024], kT[32:64, 128 * t : 128 * t + 128],
                        qT[32:64, qsl], start=True, stop=True, tile_position=(32, 0),
                    )
                    # interleaved remaining projections during supergroup 0
                    if g == 0 and t < 15:
                        v_proj(t + 1)
                    if t == 12 and g < NSG - 1:
                        q_proj(g + 1)
                # attend two sets behind S/exp
                if si - 2 in ets:
                    et_p, g_p, t_p = ets.pop(si - 2)
                    if t_p == 0:
                        for qh in range(4):
                            pa_tiles[(g_p, qh)] = ps_att.tile(
                                [128, 512], f32, tag="a", name=f"pa{g_p}_{qh}"
                            )
                    for qh in range(4):
                        pa = pa_tiles[(g_p, qh)]
                        for ci in range(2):
                            nc.tensor.matmul(
                                pa[:, 0:257],
                                et_p[:, 512 * ci + 128 * qh : 512 * ci + 128 * qh + 128],
                                vsb[:, 2 * t_p + ci, 0:257],
                                start=(t_p == 0 and ci == 0),
                                stop=(t_p == NSETS - 1 and ci == 1),
                            )
                    if t_p == NSETS - 1:
                        epi_pending = g_p
                if si < total:
                    et = expp.tile([128, 1024], bf, tag="e")
                    nc.scalar.activation(et[:], pst[:], Exp, bias=ebias[:])
                    ets[si] = (et, g, t)
                # epilogue after this slot's exp
                if epi_pending is not None:
                    epilogue_batch(
                        epi_pending,
                        [pa_tiles.pop((epi_pending, qh)) for qh in range(4)],
                    )
                    epi_pending = None

    nc.compile()
    return nc


def _get_compiled(use_bias: bool):
    key = bool(use_bias)
    if key not in _compiled_cache:
        _compiled_cache[key] = _build(use_bias)
    return _compiled_cache[key]


def _prep(x, wq, bq, wk, bk, wv, bv, wo, bo):
    xf = np.ascontiguousarray(np.asarray(x, dtype=np.float32)).reshape(B, N, C)
    wq = np.asarray(wq, np.float32)
    bq = np.asarray(bq, np.float32)
    wk = np.asarray(wk, np.float32)
    bk = np.asarray(bk, np.float32)
    wv = np.asarray(wv, np.float32)
    bv = np.asarray(bv, np.float32)
    wo = np.asarray(wo, np.float32)
    bo = np.asarray(bo, np.float32)

    use_bias = not (np.all(bq == 0) and np.all(bk == 0) and np.all(bv == 0))

    scale = np.float32(1.0 / np.sqrt(np.float32(D)))
    wqs = wq * scale
    # lhsT tiles: [2 c-halves, 128, 64] with d replicated 2x along columns
    wqp = np.ascontiguousarray(np.tile(wqs.reshape(2, 128, D), (1, 1, 2))).astype(BF16)
    wkp = np.ascontiguousarray(np.tile(wk.reshape(2, 128, D), (1, 1, 2))).astype(BF16)
    if use_bias:
        # scores row-fold: r_k = x_k @ u, u = scale * (wk @ bq); exp(r) scales
        # key k's et column (bk and bq*bk terms drop out of softmax).
        u = (wk @ (bq * scale)).astype(np.float32)  # [C]
        wvx = np.concatenate([wv, u[:, None]], axis=1)  # [C, 257]
        wvp = np.ascontiguousarray(wvx.reshape(2, 128, 257)).astype(BF16)
    else:
        wvp = np.ascontiguousarray(wv.reshape(2, 128, 256)).astype(BF16)
    wop = np.ascontiguousarray(wo.reshape(2, 128, 256)).astype(BF16)

    in_maps = []
    for core in range(NCORES):
        b, h = divmod(core, 2)
        if h == 0:
            xo = xf[b]
        else:
            xo = np.concatenate([xf[b, NQ:], xf[b, :NQ]], 0)
        xT = np.ascontiguousarray(xo.T.reshape(2, 128, N)).astype(BF16)
        in_maps.append(
            {"xT16": xT, "wqp": wqp, "wkp": wkp, "wvp": wvp, "wop": wop}
        )
    # host residual fold: out += x + (bv @ wo + bo)
    resid_const = (bv.astype(np.float64) @ wo.astype(np.float64)).astype(np.float32) + bo
    return in_maps, use_bias, xf, resid_const


def _gather(results, xf, resid_const):
    out = np.empty((B, N, C), np.float32)
    for core in range(NCORES):
        b, h = divmod(core, 2)
        out[b, NQ * h : NQ * (h + 1)] = results[core]["out"]
    out += xf
    out += resid_const[None, None, :]
    return out.reshape(B, HH, WW, C)


def kernel(x, wq, bq, wk, bk, wv, bv, wo, bo):
    from concourse.bass_utils import run_bass_kernel_spmd

    in_maps, use_bias, xf, resid_const = _prep(x, wq, bq, wk, bk, wv, bv, wo, bo)
    nc = _get_compiled(use_bias)
    res = run_bass_kernel_spmd(nc, in_maps, core_ids=list(range(NCORES)))
    return _gather(res.results, xf, resid_const)


def _ensure_ntff_hook():
    """The agent image's antenv stub lacks axon_hooks; synthesize it so
    run_bass_kernel_spmd(trace=True) can NTFF-profile via libaxon_pjrt."""
    import types

    try:
        from antenv.axon_hooks import get_axon_ntff_profile_hook  # noqa: F401
        return
    except ImportError:
        pass
    import antenv
    from trn_agent_boot.trn_boot import _ntff_profile_via_ctypes

    mod = types.ModuleType("antenv.axon_hooks")
    state = {"h": _ntff_profile_via_ctypes("/opt/axon/libaxon_pjrt.so")}
    mod.get_axon_ntff_profile_hook = lambda: state["h"]
    mod.set_axon_ntff_profile_hook = lambda h: state.__setitem__("h", h)
    sys.modules["antenv.axon_hooks"] = mod
    antenv.axon_hooks = mod


def run_traced(inputs, **kw):
    """For test.py: run with NTFF profiling; returns (output, BassKernelResults)."""
    from concourse.bass_utils import run_bass_kernel_spmd

    _ensure_ntff_hook()

    in_maps, use_bias, xf, resid_const = _prep(**inputs)
    nc = _get_compiled(use_bias)
    res = run_bass_kernel_spmd(nc, in_maps, core_ids=list(range(NCORES)), trace=True, **kw)
    return _gather(res.results, xf, resid_const), res
